# revision 57
# baseline (speedup 1.0000x reference)
"""AxialCrossMamba Trainium2 kernel.

Sharding: 8 cores = 4 directions x 2 batch-halves. Each core runs one
direction's Mamba block (its own weights) over two batches. Host does the
direction permutations (row/col/diag/anti, c-major [C, L] token layouts),
and the final 4-direction sigmoid gate.

Per-core schedule (j = batch index on this core):
  A(j): in-proj matmul (PE) -> causal depthwise conv via diagonal PE
        weights -> fused Silu (ACT). z-gate half: Silu -> z_scr (DRAM).
  C(j): x-proj matmul -> B/C rows to bc_scr (DRAM), dt softplus via
        exp+ln (single ACT table).
  D(j): selective scan per 512-token chunk: a = exp(dt*A) bf16 (ACT),
        B/C broadcast to 128 partitions by DMA (quarter tiles, ring-4
        prefetch), b = u*B (DVE 2x), tensor_tensor_scan over flattened
        (s,t) with boundary-reset columns (DVE; ~2 cyc/elem recurrence
        limit), h*C + pairwise tree reduce (DVE), out-proj folded in
        per chunk (PE).
  j=1's A/Z phases are staged through DRAM (xs_stg) and emitted between
  j=0's scan chunks so PE/ACT fill the scan phase. Softplus is emitted
  per token-half so each batch's scan starts after only half of it.
  Conv chunks trail in-proj chunks by one inside each m-block (the conv
  window peeks 3 columns ahead), keeping PE and DVE co-busy in phase A.
  Boundary-column zeros of the a tiles are written only while the pool
  slots are fresh (they are never overwritten later). When D_skip == 1
  (host-detected, with a general fallback), the y-skip uses a 2x-rate
  plain add instead of scalar_tensor_tensor.

Notes from tuning on hw: tensor_tensor_scan is DVE-only (Pool ISA
rejects it) and runs at ~2.1 ns/elem regardless of dtypes; concurrent
GPSIMD tensor ops slow DVE ~2x (SBUF contention), so GPSIMD is left
idle; DMA-compute (CCE) supports add but not mult and its latency
cancels its savings; emission order is a tuned local optimum - small
reorderings cost 100+ us via the static scheduler.
"""

import sys

for _p in ("/opt/trn_rl_repo", "/root/.axon_site/_ro/trn_rl_repo"):
    if _p not in sys.path:
        sys.path.insert(0, _p)

from contextlib import ExitStack

import numpy as np
import ml_dtypes

import concourse.bass as bass
from concourse import bacc
import concourse.mybir as mybir
import concourse.tile as tile
from concourse.bass_utils import run_bass_kernel_spmd

BF16 = ml_dtypes.bfloat16

# Problem constants
B_, C_, H_, W_ = 4, 192, 64, 64
L = H_ * W_          # 4096 tokens
DS, DC = 16, 4       # d_state, d_conv
DI = 2 * C_          # 384 d_inner
DTR = (C_ + 15) // 16  # 12 dt_rank
NB = 2               # batches per core
ND = DI // 128       # 3 d-blocks
N_CORES = 8

AF = mybir.ActivationFunctionType
ALU = mybir.AluOpType
FP32 = mybir.dt.float32
BF = mybir.dt.bfloat16


def build_nc(L=L, TC=512, SB=8, dsk_one=False):
    """Build the SPMD single-core program (identical on all 8 cores)."""
    nc = bacc.Bacc("TRN2", debug=False)

    # ---- DRAM I/O ----
    tokT = nc.dram_tensor("tokT", [NB, C_, L], BF, kind="ExternalInput").ap()
    Win = nc.dram_tensor("Win", [C_, 2 * DI], BF, kind="ExternalInput").ap()
    convd = nc.dram_tensor("convd", [ND, DC, 128, 128], BF, kind="ExternalInput").ap()
    convb = nc.dram_tensor("convb", [DI, 1], FP32, kind="ExternalInput").ap()
    Wx = nc.dram_tensor("Wx", [DI, 96], BF, kind="ExternalInput").ap()
    Wdt = nc.dram_tensor("Wdt", [DTR, DI], BF, kind="ExternalInput").ap()
    bdt = nc.dram_tensor("bdt", [DI, 1], FP32, kind="ExternalInput").ap()
    Acoef = nc.dram_tensor("Acoef", [DI, DS], FP32, kind="ExternalInput").ap()
    Dsk = nc.dram_tensor("Dsk", [DI, 1], FP32, kind="ExternalInput").ap()
    Wout = nc.dram_tensor("Wout", [DI, C_], BF, kind="ExternalInput").ap()
    outT = nc.dram_tensor("outT", [NB, C_, L], FP32, kind="ExternalOutput").ap()
    # scratch
    z_scr = nc.dram_tensor("z_scr", [NB, ND, 128, L], BF, kind="Internal").ap()
    xs_stg = nc.dram_tensor("xs_stg", [ND, 128, L], BF, kind="Internal").ap()
    bc_scr = nc.dram_tensor("bc_scr", [NB, 2, L // TC, DS * TC], BF, kind="Internal").ap()

    io = dict(tokT=tokT, Win=Win, convd=convd, convb=convb, Wx=Wx, Wdt=Wdt,
              bdt=bdt, Acoef=Acoef, Dsk=Dsk, Wout=Wout, outT=outT,
              z_scr=z_scr, bc_scr=bc_scr, xs_stg=xs_stg)
    with tile.TileContext(nc) as tc:
        with ExitStack() as ctx:
            _emit(ctx, tc, nc, io, L=L, TC=TC, SB=SB, dsk_one=dsk_one)
    nc.compile()
    return nc


def _emit(ctx, tc, nc, io, *, L, TC, SB, dsk_one):
    tokT, Win, convd, convb, Wx, Wdt, bdt = (
        io["tokT"], io["Win"], io["convd"], io["convb"], io["Wx"], io["Wdt"],
        io["bdt"])
    Acoef, Dsk, Wout, outT = io["Acoef"], io["Dsk"], io["Wout"], io["outT"]
    z_scr, bc_scr, xs_stg = io["z_scr"], io["bc_scr"], io["xs_stg"]

    P = 128
    NCH = L // TC          # t-chunks
    NSB = DS // SB         # s-blocks
    NN = max(1, L // 512)  # matmul n-chunks
    NSZ = L // NN

    # ---- pools ----
    wp = ctx.enter_context(tc.tile_pool(name="weights", bufs=1))
    big = ctx.enter_context(tc.tile_pool(name="big", bufs=4))    # bf16 [128,L]
    fxf = ctx.enter_context(tc.tile_pool(name="fxf", bufs=2))    # fp32 esp
    xsp = ctx.enter_context(tc.tile_pool(name="xsp", bufs=1))    # xs resident
    dtp = ctx.enter_context(tc.tile_pool(name="dtp", bufs=1))    # dt resident
    bcp = ctx.enter_context(tc.tile_pool(name="bc", bufs=4))     # B/C bcast
    abf = ctx.enter_context(tc.tile_pool(name="abf", bufs=2))    # fp32 scan a
    hbf = ctx.enter_context(tc.tile_pool(name="hbf", bufs=2))    # bf16 scan h
    bcls = ctx.enter_context(tc.tile_pool(name="bcls", bufs=3))  # b_ / hcm
    sm = ctx.enter_context(tc.tile_pool(name="small", bufs=2))
    smE = ctx.enter_context(tc.tile_pool(name="smallE", bufs=2))
    pp = ctx.enter_context(tc.tile_pool(name="psum", bufs=2, space="PSUM"))
    pp2 = ctx.enter_context(tc.tile_pool(name="psum2", bufs=2, space="PSUM"))
    ppE = ctx.enter_context(tc.tile_pool(name="psumE", bufs=1, space="PSUM"))

    # ---- j0 token loads first (unblocks the first in-proj ASAP) ----
    tok0_0 = big.tile([P, L], BF, tag="big", name="tok0_0")
    tok1_0 = big.tile([C_ - P, L], BF, tag="big", name="tok1_0")
    nc.sync.dma_start(tok0_0[:], tokT[0, 0:P, :])
    nc.sync.dma_start(tok1_0[:], tokT[0, P:C_, :])

    # ---- load weights ----
    win0 = wp.tile([P, 2 * DI], BF, tag="win0")
    win1 = wp.tile([C_ - P, 2 * DI], BF, tag="win1")
    nc.sync.dma_start(win0[:], Win[0:P, :])
    nc.sync.dma_start(win1[:], Win[P:C_, :])
    wdt_full = wp.tile([DTR, DI], BF, tag="wdt")
    nc.sync.dma_start(wdt_full[:], Wdt[:])
    wxs, cw3, cb3, bdt3, ac3, dsk3, wo3 = [], [], [], [], [], [], []
    for db in range(ND):
        r = slice(db * P, (db + 1) * P)
        w1 = wp.tile([P, 96], BF, tag=f"wx{db}")
        nc.sync.dma_start(w1[:], Wx[r, :]); wxs.append(w1)
        wconv = []
        for k in range(DC):
            wck = wp.tile([P, P], BF, tag=f"cw{db}_{k}", name=f"cw{db}_{k}")
            nc.sync.dma_start(wck[:], convd[db, k])
            wconv.append(wck)
        cw3.append(wconv)
        w3 = wp.tile([P, 1], FP32, tag=f"cb{db}")
        nc.sync.dma_start(w3[:], convb[r, :]); cb3.append(w3)
        w4 = wp.tile([P, 1], FP32, tag=f"bdt{db}")
        nc.sync.dma_start(w4[:], bdt[r, :]); bdt3.append(w4)
        w5 = wp.tile([P, DS], FP32, tag=f"ac{db}")
        nc.sync.dma_start(w5[:], Acoef[r, :]); ac3.append(w5)
        w6 = wp.tile([P, 1], FP32, tag=f"dsk{db}")
        nc.sync.dma_start(w6[:], Dsk[r, :]); dsk3.append(w6)
        w7 = wp.tile([P, C_], BF, tag=f"wo{db}")
        nc.sync.dma_start(w7[:], Wout[r, :]); wo3.append(w7)

    hcarry = {}
    for db in range(ND):
        for sb in range(NSB):
            hcarry[(db, sb)] = sm.tile([P, SB, 1], BF, name=f"carry{db}{sb}",
                                       tag=f"carry{db}_{sb}", bufs=1)

    def emit_tok(j):
        tok0 = big.tile([P, L], BF, tag="big", name="tok0")
        tok1 = big.tile([C_ - P, L], BF, tag="big", name="tok1")
        nc.sync.dma_start(tok0[:], tokT[j, 0:P, :])
        nc.sync.dma_start(tok1[:], tokT[j, P:C_, :])
        return tok0, tok1

    def emit_a(j, toks, staged, fast_copy=False, m_list=None):
        """in-proj (x half) + conv + silu. staged: xs -> DRAM xs_stg."""
        tok0, tok1 = toks
        xs_l = []
        for m in (range(ND) if m_list is None else m_list):
            xi = big.tile([P, L + DC], BF, tag="big", name="xi")
            nc.scalar.memzero(xi[:, 0:DC])
            mm = slice(m * P, (m + 1) * P)
            db = m
            if staged:
                x_ = big.tile([P, L], BF, tag="big", name="xstg")
            else:
                x_ = xsp.tile([P, L], BF, tag=f"xs{db}", name="xres")
            # interleave: in-proj chunk n, then conv chunk n-1 (the conv
            # window reads 3 columns into chunk n, so it trails by one)
            for n in range(NN + 1):
                if n < NN:
                    ns = slice(n * NSZ, (n + 1) * NSZ)
                    ps = pp.tile([P, NSZ], FP32, tag="ps")
                    nc.tensor.matmul(ps[:], win0[:, mm], tok0[:, ns],
                                     start=True, stop=False)
                    nc.tensor.matmul(ps[:], win1[:, mm], tok1[:, ns],
                                     start=False, stop=True)
                    dst = xi[:, DC + n * NSZ: DC + (n + 1) * NSZ]
                    if fast_copy:
                        nc.vector.tensor_copy(dst, ps[:])
                    else:
                        nc.scalar.copy(dst, ps[:])
                nc_ = n - 1
                if nc_ >= 0:
                    ns = slice(nc_ * NSZ, (nc_ + 1) * NSZ)
                    psc = pp.tile([P, NSZ], FP32, tag="psc")
                    for k in range(DC):
                        nc.tensor.matmul(
                            psc[:], cw3[db][k][:],
                            xi[:, 1 + k + nc_ * NSZ: 1 + k + nc_ * NSZ + NSZ],
                            start=(k == 0), stop=(k == DC - 1))
                    nc.scalar.activation(x_[:, ns], psc[:], AF.Silu,
                                         bias=cb3[db])
            if staged:
                nc.sync.dma_start(xs_stg[db], x_[:])
            xs_l.append(x_)
        return xs_l

    def emit_z(j, toks, n_lo=0, n_hi=None):
        """in-proj z half + silu -> z_scr (chunk range [n_lo, n_hi))."""
        tok0, tok1 = toks
        for n in range(n_lo, NN if n_hi is None else n_hi):
            ns = slice(n * NSZ, (n + 1) * NSZ)
            for m in range(ND, 2 * DI // P):
                mm = slice(m * P, (m + 1) * P)
                ps = pp.tile([P, NSZ], FP32, tag="ps")
                nc.tensor.matmul(ps[:], win0[:, mm], tok0[:, ns],
                                 start=True, stop=False)
                nc.tensor.matmul(ps[:], win1[:, mm], tok1[:, ns],
                                 start=False, stop=True)
                zt = smE.tile([P, NSZ], BF, tag="ztmp", bufs=1)
                nc.scalar.activation(zt[:], ps[:], AF.Silu)
                nc.sync.dma_start(z_scr[j, m - ND, :, ns], zt[:])

    def emit_reload_xs():
        xs_l = []
        for db in range(ND):
            x_ = big.tile([P, L], BF, tag="big", name="xrel")
            nc.sync.dma_start(x_[:], xs_stg[db])
            xs_l.append(x_)
        return xs_l

    def emit_c_bc(j, xs_l):
        """x-proj (B/C to bc_scr, dt-low to dtl)."""
        dtl = sm.tile([DTR, L], BF, tag="dtl", bufs=1, name="dtl")
        for n in range(NN):
            ns = slice(n * NSZ, (n + 1) * NSZ)
            psd = pp2.tile([96, NSZ], FP32, tag="psd")
            for db in range(ND):
                nc.tensor.matmul(psd[:], wxs[db][:], xs_l[db][:, ns],
                                 start=(db == 0), stop=(db == ND - 1))
            nc.scalar.copy(dtl[:, ns], psd[0:DTR, :])
            bt = smE.tile([DS, NSZ], BF, tag="bct", name="bt")
            ct = smE.tile([DS, NSZ], BF, tag="bct", name="ct")
            nc.vector.tensor_copy(bt[:], psd[32:32 + DS, :])
            nc.vector.tensor_copy(ct[:], psd[64:64 + DS, :])
            for r in range(max(1, NSZ // TC)):
                rs = slice(r * TC, (r + 1) * TC)
                nc.sync.dma_start(
                    bc_scr[j, 0, n * (NSZ // TC) + r]
                    .rearrange("(s t) -> s t", s=DS), bt[:, rs])
                nc.sync.dma_start(
                    bc_scr[j, 1, n * (NSZ // TC) + r]
                    .rearrange("(s t) -> s t", s=DS), ct[:, rs])
        return dtl

    def emit_softplus_half(dtl, dts, h):
        """softplus for all 3 d-blocks over token half h -> dts tiles."""
        LH = L // 2
        esps = {}

        def sp_exps(db):
            esp = fxf.tile([P, LH], BF, tag="xf", name="esp")
            for n2 in range(NN // 2):
                n = h * (NN // 2) + n2
                ns = slice(n * NSZ, (n + 1) * NSZ)
                psm = pp.tile([P, NSZ], FP32, tag="ps")
                nc.tensor.matmul(psm[:], wdt_full[:, db * P:(db + 1) * P],
                                 dtl[:, ns], start=True, stop=True)
                nc.scalar.activation(esp[:, n2 * NSZ:(n2 + 1) * NSZ],
                                     psm[:], AF.Exp, bias=bdt3[db])
            esps[db] = esp

        def sp_ln(db):
            if dts[db] is None:
                dts[db] = dtp.tile([P, L], BF, tag=f"dt{db}", name="d_")
            nc.scalar.activation(dts[db][:, h * LH:(h + 1) * LH],
                                 esps[db][:], AF.Ln, bias=1.0)

        # cluster exps before lns (esp ring is 2, so Ln0 precedes exps2)
        sp_exps(0); sp_exps(1); sp_ln(0); sp_ln(1); sp_exps(2); sp_ln(2)

    SQ = SB // 2   # broadcast quarter size

    def emit_d(j, xs_l, dtf, ch_lo, ch_hi):
        """selective scan + out-proj for chunks [ch_lo, ch_hi)."""
        for ch in range(ch_lo, ch_hi):
            cs = slice(ch * TC, (ch + 1) * TC)
            # hoisted B/C broadcasts (quarter tiles), shared by all 3 d-blocks
            breps, creps = [], []
            for q in range(DS // SQ):
                brep = bcp.tile([P, SQ, TC], BF, tag="bc", name="brep")
                crep = bcp.tile([P, SQ, TC], BF, tag="bc", name="crep")
                nc.sync.dma_start(
                    brep[:],
                    bc_scr[j, 0, ch, q * SQ * TC:(q + 1) * SQ * TC]
                    .rearrange("(s t) -> s t", s=SQ)
                    .unsqueeze(0).broadcast_to((P, SQ, TC)))
                nc.sync.dma_start(
                    crep[:],
                    bc_scr[j, 1, ch, q * SQ * TC:(q + 1) * SQ * TC]
                    .rearrange("(s t) -> s t", s=SQ)
                    .unsqueeze(0).broadcast_to((P, SQ, TC)))
                breps.append(brep); creps.append(crep)

            uchs = []
            for db in range(ND):
                uch = sm.tile([P, TC], BF, tag=f"uch{db}", bufs=2, name="uch")
                nc.vector.tensor_tensor(uch[:], dtf[db][:, cs],
                                        xs_l[db][:, cs], ALU.mult)
                uchs.append(uch)
            yaccs = [sm.tile([P, TC], BF, tag=f"yacc{db}", bufs=1,
                             name=f"yacc{db}") for db in range(ND)]
            for sb in range(NSB):
                for db in range(ND):
                    uv = uchs[db][:].unsqueeze(1).broadcast_to((P, SQ, TC))
                    a_ = abf.tile([P, SB, TC + 1], BF, tag="a", name="a_")
                    if a_fresh[0] > 0:
                        # zero the boundary column only while the pool slot
                        # is fresh; later generations inherit the zeros
                        # (exps never write column 0)
                        nc.vector.memset(a_[:, :, 0:1], 0.0)
                        a_fresh[0] -= 1
                    for s8 in range(SB):
                        s = sb * SB + s8
                        nc.scalar.activation(a_[:, s8, 1:], dtf[db][:, cs],
                                             AF.Exp, scale=ac3[db][:, s:s + 1])
                    b_ = bcls.tile([P, SB, TC + 1], BF, tag="bcls", name="b_")
                    for hq in range(2):
                        hs = slice(hq * SQ, (hq + 1) * SQ)
                        nc.vector.tensor_tensor(b_[:, hs, 1:], uv,
                                                breps[sb * 2 + hq][:],
                                                ALU.mult)
                    if ch == 0:
                        nc.vector.memset(b_[:, :, 0:1], 0.0)
                    else:
                        nc.vector.tensor_copy(b_[:, :, 0:1],
                                              hcarry[(db, sb)][:])
                    h_ = hbf.tile([P, SB, TC + 1], BF, tag="h", name="h_")
                    nc.vector.tensor_tensor_scan(
                        h_[:].rearrange("p s t -> p (s t)"),
                        a_[:].rearrange("p s t -> p (s t)"),
                        b_[:].rearrange("p s t -> p (s t)"),
                        0.0, ALU.mult, ALU.add)
                    nc.vector.tensor_copy(hcarry[(db, sb)][:],
                                          h_[:, :, TC:TC + 1])
                    hcm = bcls.tile([P, SB, TC], BF, tag="bcls", name="hcm")
                    for hq in range(2):
                        hs = slice(hq * SQ, (hq + 1) * SQ)
                        nc.vector.tensor_tensor(hcm[:, hs, :],
                                                h_[:, hs, 1:],
                                                creps[sb * 2 + hq][:],
                                                ALU.mult)
                    # pairwise tree-sum over the SB states (contiguous, 2x)
                    t2 = sm.tile([P, SB // 2, TC], BF, tag="t2", bufs=1,
                                 name="t2")
                    nc.vector.tensor_tensor(t2[:], hcm[:, 0:SB // 2, :],
                                            hcm[:, SB // 2:SB, :], ALU.add)
                    t4 = sm.tile([P, SB // 4, TC], BF, tag="t4", bufs=1,
                                 name="t4")
                    nc.vector.tensor_tensor(t4[:], t2[:, 0:SB // 4, :],
                                            t2[:, SB // 4:SB // 2, :], ALU.add)
                    if sb == 0:
                        nc.vector.tensor_tensor(yaccs[db][:], t4[:, 0, :],
                                                t4[:, 1, :], ALU.add)
                    else:
                        ysb = sm.tile([P, TC], BF, tag="ysb", bufs=1,
                                      name="ysb")
                        nc.vector.tensor_tensor(ysb[:], t4[:, 0, :],
                                                t4[:, 1, :], ALU.add)
                        nc.vector.tensor_tensor(yaccs[db][:], yaccs[db][:],
                                                ysb[:], ALU.add)
            y0s = []
            for db in range(ND):
                # y = ys + xs*D -> bf16 (plain add at 2x when D == 1)
                y0 = sm.tile([P, TC], BF, tag=f"y0_{db}", bufs=1, name="y0")
                if dsk_one:
                    nc.vector.tensor_tensor(y0[:], xs_l[db][:, cs],
                                            yaccs[db][:], ALU.add)
                else:
                    nc.vector.scalar_tensor_tensor(y0[:], xs_l[db][:, cs],
                                                   dsk3[db][:], yaccs[db][:],
                                                   ALU.mult, ALU.add)
                y0s.append(y0)
            # ---- out-proj for this chunk (folded phase E) ----
            ygs = []
            for db in range(ND):
                zch = smE.tile([P, TC], BF, tag="ze", bufs=3, name="zch")
                nc.sync.dma_start(zch[:], z_scr[j, db, :, cs])
                ytg = smE.tile([P, TC], BF, tag="ye", bufs=2, name="ytg")
                nc.vector.tensor_tensor(ytg[:], y0s[db][:], zch[:], ALU.mult)
                ygs.append(ytg)
            for m in range(2):
                msz = P if m == 0 else C_ - P
                mm = slice(m * P, m * P + msz)
                pso = ppE.tile([msz, TC], FP32, tag=f"pso{m}", bufs=1,
                               name="pso")
                for db in range(ND):
                    nc.tensor.matmul(pso[:], wo3[db][:, mm], ygs[db][:],
                                     start=(db == 0), stop=(db == ND - 1))
                ot = smE.tile([msz, TC], FP32, tag="oe", bufs=1, name="ot")
                nc.scalar.copy(ot[:], pso[:])
                nc.sync.dma_start(outT[j, mm, cs], ot[:])

    a_fresh = [4]   # 2x abf pool bufs: covers both slots in any assignment order
    # interleaved schedule: j1's A/Z/reload/x-proj hide under j0's scan
    toks0 = (tok0_0, tok1_0)
    xs0 = emit_a(0, toks0, staged=False, fast_copy=True)
    dtl0 = emit_c_bc(0, xs0)
    dtf0 = [None, None, None]
    emit_softplus_half(dtl0, dtf0, 0)
    emit_z(0, toks0, 0, 1)
    emit_d(0, xs0, dtf0, 0, 1)
    emit_z(0, toks0, 1, 3)
    emit_softplus_half(dtl0, dtf0, 1)
    emit_d(0, xs0, dtf0, 1, 2)
    emit_d(0, xs0, dtf0, 2, 3)
    emit_z(0, toks0, 3, None)
    toks1 = emit_tok(1)
    emit_a(1, toks1, staged=True, m_list=[0])
    emit_d(0, xs0, dtf0, 3, 4)
    emit_a(1, toks1, staged=True, m_list=[1])
    emit_d(0, xs0, dtf0, 4, 5)
    emit_a(1, toks1, staged=True, m_list=[2])
    emit_d(0, xs0, dtf0, 5, 6)
    emit_z(1, toks1)
    xs1 = emit_reload_xs()
    emit_d(0, xs0, dtf0, 6, 7)
    dtl1 = emit_c_bc(1, xs1)
    emit_d(0, xs0, dtf0, 7, NCH)
    dtf1 = [None, None, None]
    emit_softplus_half(dtl1, dtf1, 0)
    emit_d(1, xs1, dtf1, 0, 2)
    emit_softplus_half(dtl1, dtf1, 1)
    emit_d(1, xs1, dtf1, 2, NCH)


# ---------------- host side ----------------

_CACHE = {}
PROFILE = False
PROFILE_KW = {}


def _get_nc(dsk_one):
    key = f"nc{int(dsk_one)}"
    if key not in _CACHE:
        _CACHE[key] = build_nc(dsk_one=dsk_one)
    return _CACHE[key]


def _permute_toks(x, idx):
    """x: [C, H, W] fp32 -> 4 direction token maps, each [C, L] (c-major)."""
    c = x.shape[0]
    row = x.reshape(c, -1)
    col = x.transpose(0, 2, 1).reshape(c, -1)
    diag = row[:, idx]
    anti = x[:, :, ::-1].reshape(c, -1)[:, idx]
    return [row, col, diag, anti]


def _unpermute(outs, inv_idx, h, w):
    """outs: list of 4 [C, L] -> sum of un-permuted direction outputs."""
    c = outs[0].shape[0]
    row_f = outs[0].reshape(c, h, w)
    col_f = outs[1].reshape(c, w, h).transpose(0, 2, 1)
    diag_f = outs[2][:, inv_idx].reshape(c, h, w)
    anti_f = outs[3][:, inv_idx].reshape(c, h, w)[:, :, ::-1]
    return row_f + col_f + diag_f + anti_f


def _pack_convd(cw):
    """Per d-block, per tap: diag(conv_w[:, k]) as bf16 PE weights."""
    out = np.zeros((ND, DC, 128, 128), np.float32)
    for db in range(ND):
        for k in range(DC):
            np.fill_diagonal(out[db, k], cw[db * 128:(db + 1) * 128, k])
    return out.astype(BF16)


def _pack_wx(wx):
    """Pad W_x columns so dt/B/C rows land at PSUM partitions 0/32/64."""
    out = np.zeros((DI, 96), np.float32)
    out[:, 0:DTR] = wx[:, 0:DTR]
    out[:, 32:32 + DS] = wx[:, DTR:DTR + DS]
    out[:, 64:64 + DS] = wx[:, DTR + DS:]
    return out.astype(BF16)


def kernel(x, W_in, conv_w, conv_b, W_x, W_dt, b_dt, A_log, D_skip, W_out,
           idx, inv_idx):
    x = np.asarray(x, np.float32)
    idx = np.asarray(idx, np.int32)
    inv_idx = np.asarray(inv_idx, np.int32)
    A = -np.exp(np.asarray(A_log, np.float32))        # [4, DI, DS]
    conv_b = np.asarray(conv_b, np.float32)
    b_dt = np.asarray(b_dt, np.float32)
    D_skip = np.asarray(D_skip, np.float32)

    nc = _get_nc(bool(np.allclose(D_skip, 1.0)))
    in_maps = []
    for core in range(N_CORES):
        d = core // 2      # direction
        bh = core % 2      # batch half
        toks = np.empty((NB, C_, L), BF16)
        for jb in range(NB):
            b = bh * NB + jb
            toks[jb] = _permute_toks(x[b], idx)[d].astype(BF16)
        in_maps.append(dict(
            tokT=toks,
            Win=np.asarray(W_in[d], np.float32).astype(BF16),
            convd=_pack_convd(np.asarray(conv_w[d], np.float32)),
            convb=np.ascontiguousarray(conv_b[d].reshape(DI, 1)),
            Wx=_pack_wx(np.asarray(W_x[d], np.float32)),
            Wdt=np.asarray(W_dt[d], np.float32).astype(BF16),
            bdt=np.ascontiguousarray(b_dt[d].reshape(DI, 1)),
            Acoef=np.ascontiguousarray(A[d]),
            Dsk=np.ascontiguousarray(D_skip[d].reshape(DI, 1)),
            Wout=np.asarray(W_out[d], np.float32).astype(BF16),
        ))

    res = run_bass_kernel_spmd(nc, in_maps, list(range(N_CORES)),
                               trace=PROFILE, **PROFILE_KW)
    _CACHE["last_exec_ns"] = res.exec_time_ns
    outs = res.results

    # gather: per batch b, the 4 direction outputs live on cores d*2 + b//2
    acc = np.zeros((B_, C_, H_, W_), np.float32)
    for b in range(B_):
        bh, jb = b // NB, b % NB
        douts = [np.asarray(outs[d * 2 + bh]["outT"][jb], np.float32)
                 for d in range(4)]
        acc[b] = _unpermute(douts, inv_idx, H_, W_)
    gate = 1.0 / (1.0 + np.exp(-0.25 * acc))
    return x * gate
